# revision 33
# baseline (speedup 1.0000x reference)
"""MetaNetImageEncoder Trainium2 kernel (fp8, v6).

Data-parallel over batch: 8 samples per NeuronCore x 8 cores.

Numerics: x, W1, dW1, dW2, mixed deltas are fp8e4m3. Scales:
  phase 1:  pa = x8 @ (64 W1); pa64 = pa + 64 b1 (bf16, kept);
            ro = relu(pa64); poolb = sum_n ro  (= 64*196*pooled)
  phase 2:  MetaNet on poolb; host folds W2@mw1/(64*196) into mw1.
  phase 3:  mix = (8 c) x (8 dW1) = 64 c dW1 -> mxi2 fp8 (NO W1 fold).
            DoubleRow, contract (t, s32) = 256; output partition
            (b4, s'32), two half-batch stationaries.
  phase 4:  pf = x8 @ (64 dmix);  s = pf + pa64;
            relu(s/64 + nb1'), nb1' = coefs@db1 (b1 already in pa64).
  phase 5:  uall = pooln * (64/196 c); po8 = uall @ (64 dW2) -> /4096;
            main chain uses host-scaled W2/196.

DMA-queue economics drive the layout: each engine-issued DMA costs
~0.5us pacing + transfer on its queue, so everything is batched:
xt 1 DMA, dw1/dw2 4 icl-pair DMAs each, consts packed into 2 DMAs
(bitcast views), de-interleave 32 DMAs of [32 part, 4.6KB rows]
(the 32 descriptors of each spray across DMA engines in parallel).

Engine plan:
  sync    xt, dw1 pairs, half the de-interleave, out store.
  scalar  w1 + packed consts + act-table trigger, ph1 PSUM->pa64
          drains + 1 relu/jt, ph2 relus, 5/8 of ph3 drains, a couple
          of tail de-ints, ph4 relus, ph5 tdelt scales.
  vector  memsets, ph1 1 relu/jt + pool reduces + casts, ph2 glue,
          3/8 of ph3 drains, ph4 pa64 adds + pool reduces, half of
          uall, final adds.
  gpsimd  half the de-interleave, w2 + dw2 loads (during ph4),
          half of uall.
  tensor  warm-up dummies at t=0 (N=128), ph1 DoubleRow per-sample
          (FD 196), MetaNet interleaved per-jt, ph3 DR mixing, tiny
          nb1t/cbc matmuls in the ph3->ph4 de-interleave tail, ph4
          normal fp8 (FD 196), ph5 delta DoubleRow.
"""
import numpy as np
import ml_dtypes

import concourse.bass as bass
import concourse.mybir as mybir
import concourse.tile as tile
from concourse.vector_clock import ScopedClock
from concourse.bass_utils import run_bass_kernel_spmd

F32 = mybir.dt.float32
BF16 = mybir.dt.bfloat16
FP8 = mybir.dt.float8e4
U8 = mybir.dt.uint8
DR = mybir.MatmulPerfMode.DoubleRow
RELU = mybir.ActivationFunctionType.Relu
IDENT = mybir.ActivationFunctionType.Identity
ADD = mybir.AluOpType.add
MULT = mybir.AluOpType.mult
AXX = mybir.AxisListType.X

P = 16
D = 768
T = 8
HM = 192
NPAT = 196          # 14*14 patches
B = 64
NCORES = 8
BC = B // NCORES    # 8 samples per core
NB = BC * NPAT      # 1568
KT = D // 128       # 6 k-tiles
SC = 64.0           # fp8 weight-space scale
SCC = 8.0           # coefficient scale (SCC * (SC/SCC) = SC)

# packed-const byte offsets (per partition), all 4-aligned
CA_B1T64 = 0                      # [128, 6] f32        24B
CA_MB1T = 24                      # [128, 2] f32         8B
CA_MASK32 = 32                    # [128, 2, 32] bf16  128B
CA_MW2 = 160                      # [128, 2, 8] bf16    32B
CA_MW1 = 192                      # [128, 6, 192] bf16 2304B
CA_BYTES = 2496
CB_IEXP = 0                       # [8, 128] f32       512B
CB_MB2T = 512                     # [8, 1] f32           4B
CB_SELB = 516                     # [8, 8, 128] bf16  2048B
CB_DB1 = 2564                     # [8, 768] bf16     1536B
CB_DB2 = 4100                     # [8, 768] bf16     1536B
CB_B2R = 5636                     # [8, 768] f32      3072B
CB_BYTES = 8708

_PATCHED = False
_DEBUG_TAPS = False


def _apply_tile_patch():
    """This container's walrus allows only one sem wait per instruction;
    TileContext's exit drain attaches one wait per live semaphore. Split
    them onto standalone single-wait nops."""
    global _PATCHED
    if _PATCHED:
        return
    _PATCHED = True

    def _patched(self, tick_clock, wait_clock):
        carrier = self.nc.sync.nop(nofuse=True, hint="drain_waits")
        wait_clock.add_sem_waits(
            carrier.ins, ScopedClock({None: tick_clock.global_clock})
        )
        si = carrier.ins.sync_info
        waits = list(si.on_wait) if si else []
        if len(waits) > 1:
            carrier.ins.sync_info = mybir.SyncInfo(on_wait=[waits[0]], on_update=[])
            for w in waits[1:]:
                extra = self.nc.sync.nop(nofuse=True, hint="drain_waits")
                extra.ins.sync_info = mybir.SyncInfo(on_wait=[w], on_update=[])
        self.nc.sync.drain()
        self.nc.all_engine_barrier()
        popped = self.nc._tile_sem_poison_stack.pop()
        assert popped is self._sem_poison
        self.nc.clear_and_free_semaphores(list(self.sems.allocated().values()))
        self.nc.all_engine_barrier()

    tile.TileContext._drain_and_barrier = _patched


def _split_multi_waits(nc, max_waits: int = 1):
    """Hoist extra sem waits onto same-engine InstNoOp carriers."""
    for f in nc.m.functions:
        for blk in f.blocks:
            out = []
            for inst in blk.instructions:
                si = inst.sync_info
                if si is not None and len(si.on_wait) > max_waits:
                    waits = list(si.on_wait)
                    for i, w in enumerate(waits[:-max_waits]):
                        out.append(mybir.InstNoOp(
                            name=f"{inst.name}-w{i}",
                            sync_info=mybir.SyncInfo(on_wait=[w], on_update=[]),
                            bass_nofuse=True,
                            engine=inst.engine,
                        ))
                    inst.sync_info = mybir.SyncInfo(
                        on_wait=waits[-max_waits:], on_update=list(si.on_update)
                    )
                out.append(inst)
            blk.instructions = out


def build_kernel():
    nc = bass.Bass(target_bir_lowering=False, trn_type="TRN2")

    din = {}
    def inp(name, shape, dt):
        din[name] = nc.dram_tensor(name, shape, dt, kind="ExternalInput")
        return din[name]

    xt = inp("xt", (128, KT, NB), FP8)           # patches^T  [i_local, kt, (b,n)]
    w1 = inp("w1", (128, KT, D), FP8)            # 64*W1 [i_local, kt, j]
    w2 = inp("w2", (128, KT, D), BF16)           # W2/196 [j_local, kt, e]
    # dw1 pair-tiles: [pair, (t,s32)/2, (iclp, it, r), j]
    dw1 = inp("dw1", (4, 128, KT * 2, D), FP8)
    dw2 = inp("dw2", (4, 128, 2 * KT, D), FP8)   # 64*dW2 [pair,(j),(tp,kt),e]
    b1t64 = inp("b1t64", (128, KT), F32)
    mb1t = inp("mb1t", (128, 2), F32)
    mask32 = inp("mask32", (128, 2, 32), BF16)
    mw2 = inp("mw2", (128, 2, T), BF16)
    mw1 = inp("mw1", (128, KT, HM), BF16)
    iexp = inp("iexp", (T, 128), F32)
    mb2t = inp("mb2t", (T, 1), F32)
    selb = inp("selb", (T, T, 128), BF16)
    db1 = inp("db1", (T, D), BF16)
    db2 = inp("db2", (T, D), BF16)
    b2r = inp("b2r", (BC, D), F32)

    out = nc.dram_tensor("out", (BC, D), F32, kind="ExternalOutput")
    if _DEBUG_TAPS:
        dbg_poolb = nc.dram_tensor("dbg_poolb", (128, KT, BC), F32,
                                   kind="ExternalOutput")
        dbg_coefs = nc.dram_tensor("dbg_coefs", (T, T), F32,
                                   kind="ExternalOutput")
        dbg_pooln = nc.dram_tensor("dbg_pooln", (128, KT, BC), F32,
                                   kind="ExternalOutput")
        dbg_tdelt = nc.dram_tensor("dbg_tdelt", (BC, D), F32,
                                   kind="ExternalOutput")
        dbg_mxc = nc.dram_tensor("dbg_mxc", (128, BC, KT, D), FP8,
                                 kind="ExternalOutput")
        dbg_pa64 = nc.dram_tensor("dbg_pa64", (128, KT, BC, NPAT), BF16,
                                  kind="ExternalOutput")

    with tile.TileContext(nc) as tc:
        with (
            tc.tile_pool(name="big", bufs=1) as big,
            tc.tile_pool(name="sm", bufs=1) as sm,
            tc.tile_pool(name="dwp", bufs=5) as dwp,
            tc.tile_pool(name="scr", bufs=2) as scr,
        ):
            # ---------- scratch memsets on DVE (GPS is ~14ns/elem) ----------
            warm_sb = sm.tile([128, 128], FP8, tag="warm")
            nc.vector.memset(warm_sb[:], 0.0)
            uall = sm.tile([128, T, KT, 2 * BC], FP8, tag="uall")
            nc.vector.memset(uall[:], 0.0)
            zeros_sb = sm.tile([128, 2, 2, 384], FP8, tag="zeros")
            nc.vector.memset(zeros_sb[:], 0.0)

            # ---------- bulk loads ----------
            # w1 + xt + dw1 all FIFO on sync: dw1 transfers must NOT
            # contend with xt/w1 for DMA fabric (ph1 start is gated on
            # xt+w1; dw1 is only needed from ~25us).
            w1_sb = big.tile([128, KT, D], FP8, tag="w1")
            nc.sync.dma_start(w1_sb[:], w1[:])
            b1t64_sb = sm.tile([128, KT], F32, tag="b1t64")
            nc.scalar.dma_start(b1t64_sb[:], b1t64[:])
            mb1t_sb = sm.tile([128, 2], F32, tag="mb1t")
            nc.scalar.dma_start(mb1t_sb[:], mb1t[:])
            mask32_sb = sm.tile([128, 2, 32], BF16, tag="mask32")
            nc.gpsimd.dma_start(mask32_sb[:], mask32[:])
            mw2_sb = sm.tile([128, 2, T], BF16, tag="mw2")
            nc.gpsimd.dma_start(mw2_sb[:], mw2[:])
            mw1_sb = sm.tile([128, KT, HM], BF16, tag="mw1")
            nc.gpsimd.dma_start(mw1_sb[:], mw1[:])
            iexp_sb = sm.tile([T, 128], F32, tag="iexp")
            nc.gpsimd.dma_start(iexp_sb[:], iexp[:])
            mb2t_sb = sm.tile([T, 1], F32, tag="mb2t")
            nc.gpsimd.dma_start(mb2t_sb[:], mb2t[:])
            selb_sb = sm.tile([T, T, 128], BF16, tag="selb")
            nc.gpsimd.dma_start(selb_sb[:], selb[:])
            db1_sb = sm.tile([T, D], BF16, tag="db1")
            nc.gpsimd.dma_start(db1_sb[:], db1[:])
            db2_sb = sm.tile([T, D], BF16, tag="db2")
            nc.gpsimd.dma_start(db2_sb[:], db2[:])
            b2r_sb = sm.tile([BC, D], F32, tag="b2r")
            nc.gpsimd.dma_start(b2r_sb[:], b2r[:])

            # dummy activation: pulls the 1.3us ACT table load into the
            # DMA window instead of mid-phase-1
            acttrig = sm.tile([128, 2], BF16, tag="acttrig")
            nc.scalar.activation(acttrig[:], mb1t_sb[:], RELU)

            # sync: xt then dw1 pair-tiles
            xt_sb = big.tile([128, KT, NB], FP8, tag="xt")
            for g in range(3):
                nc.sync.dma_start(xt_sb[:, 2 * g:2 * g + 2, :],
                                  xt[:, 2 * g:2 * g + 2, :])

            pa64 = big.tile([128, KT, BC, NPAT], BF16, tag="pa64")
            poolb = sm.tile([128, KT, BC], F32, tag="poolb")
            poolb_bf = sm.tile([128, KT, BC], BF16, tag="poolbbf")
            pooln = sm.tile([128, KT, BC], F32, tag="pooln")

            dw1_tiles = []
            # ---------- phase 1: base pass (fp8 DoubleRow, per-sample) ----
            # + per-jt MetaNet layer-1 accumulation interleaved.
            mh0 = sm.tile([128, T], BF16, tag="mh0")
            mh1 = sm.tile([64, T], BF16, tag="mh1")
            with tc.tile_pool(name="psMH", bufs=1, space="PSUM") as psMH:
                # warm-up: N=128 dummies keep PE busy from ~1us so HAM
                # un-throttles before real work arrives.
                pw = psMH.tile([128, 128], F32, tag="mh0")
                for _ in range(24):
                    nc.tensor.matmul(pw[:], warm_sb[:], warm_sb[:],
                                     start=True, stop=True)

                pm0 = psMH.tile([128, T], F32, tag="mh0")
                pm1 = psMH.tile([64, T], F32, tag="mh1")
                with tc.tile_pool(name="psA", bufs=3, space="PSUM") as psA:
                    for jt in range(KT):
                        phs = []   # two [128, 4, 256] psum tiles (2 banks ea)
                        for h in range(2):
                            ph = psA.tile([128, 4, 256], F32, tag="a")
                            phs.append(ph)
                        # one accumulation group per PSUM bank (2 samples
                        # share a bank; per-element has_written handles it)
                        for g in range(3):
                            for b8 in range(BC):
                                nc.tensor.matmul(
                                    phs[b8 // 4][:, b8 % 4, 0:196],
                                    w1_sb[:, 2 * g:2 * g + 2,
                                          jt * 128:(jt + 1) * 128],
                                    xt_sb[:, 2 * g:2 * g + 2,
                                          b8 * NPAT:(b8 + 1) * NPAT],
                                    start=(g == 0 and b8 % 2 == 0),
                                    stop=(g == 2 and b8 % 2 == 1),
                                    perf_mode=DR)
                        ro = scr.tile([128, BC, NPAT], BF16, tag="ro")
                        for h in range(2):
                            # ACT: PSUM -> pa64 (pre-relu base + 64*b1)
                            nc.scalar.activation(
                                pa64[:, jt, 4 * h:4 * h + 4, :],
                                phs[h][:, :, 0:196], IDENT,
                                bias=b1t64_sb[:, jt:jt + 1])
                            # relu on DVE (TS-max ~0.45ns/el; ACT's table
                            # path is 2-3x slower). fp8 ro is fine: poolb
                            # feeds only the MetaNet coefficients.
                            nc.vector.tensor_scalar_max(
                                ro[:, 4 * h:4 * h + 4, :],
                                pa64[:, jt, 4 * h:4 * h + 4, :], 0.0)
                        # DVE: pool straight to bf16 (2x 16-bit path; the
                        # ~0.3% bf16-accumulation error only touches the
                        # MetaNet coefficients). Split per-half on the last
                        # jt to shorten the serial tail into the MetaNet.
                        with nc.allow_low_precision("poolb feeds MetaNet"):
                            if jt < KT - 1:
                                nc.vector.tensor_reduce(
                                    poolb_bf[:, jt, :], ro[:],
                                    axis=AXX, op=ADD)
                            else:
                                for h in range(2):
                                    nc.vector.tensor_reduce(
                                        poolb_bf[:, jt, 4 * h:4 * h + 4],
                                        ro[:, 4 * h:4 * h + 4, :],
                                        axis=AXX, op=ADD)
                        nc.tensor.matmul(pm0[:], mw1_sb[:, jt, 0:128],
                                         poolb_bf[:, jt, :],
                                         start=(jt == 0), stop=(jt == KT - 1))
                        nc.tensor.matmul(pm1[:], mw1_sb[:, jt, 128:HM],
                                         poolb_bf[:, jt, :],
                                         start=(jt == 0), stop=(jt == KT - 1))

                # dw1 loads: gated on phase-1 progress (tiny dummy write
                # into each tile corner that depends on the jt=0 drain) so
                # their 4.7MB of transfers don't steal DMA fabric from the
                # xt/w1 loads that gate phase-1 start.
                for pr in range(4):
                    t_ = dwp.tile([128, KT * 2, D], FP8, tag="dw")
                    nc.gpsimd.tensor_copy(
                        t_[0:1, 0, 0:4], pa64[0:1, 0, 0, 0:4])
                    nc.sync.dma_start(t_[:], dw1[pr])
                    dw1_tiles.append(t_)

                # ---------- phase 2: MetaNet tail ----------
                with tc.tile_pool(name="pst", bufs=2, space="PSUM") as pst:
                    def warm(n=3):
                        for _ in range(n):
                            pj = pst.tile([128, 512], F32, tag="tiny")
                            nc.tensor.matmul(
                                pj[:], w1_sb[:, 0, 0:128], xt_sb[:, 0, 0:512],
                                start=True, stop=True)

                    nc.scalar.activation(mh0[:], pm0[:], RELU,
                                         bias=mb1t_sb[:, 0:1])
                    nc.scalar.activation(mh1[:], pm1[:], RELU,
                                         bias=mb1t_sb[0:64, 1:2])
                    warm()
                    pc = pst.tile([T, T], F32, tag="tiny")
                    nc.tensor.matmul(pc[:], mw2_sb[:, 0, :], mh0[:],
                                     start=True, stop=False)
                    nc.tensor.matmul(pc[:], mw2_sb[0:64, 1, :], mh1[:],
                                     start=False, stop=True)
                    coefsT = sm.tile([T, T], F32, tag="coefsT")
                    nc.vector.tensor_scalar_add(coefsT[:], pc[:], mb2t_sb[:])
                    coefsT_bf = sm.tile([T, T], BF16, tag="coefsTbf")
                    nc.vector.tensor_copy(coefsT_bf[:], coefsT[:])
                    warm()

                    # cRep[(t,s), b] = c[t, b] replicated
                    pr2 = pst.tile([128, T], F32, tag="tiny")
                    nc.tensor.matmul(pr2[:], iexp_sb[:], coefsT[:],
                                     start=True, stop=True)
                    crep = sm.tile([128, T], F32, tag="crep")
                    nc.vector.tensor_copy(crep[:], pr2[:])

                    # DR stationaries cb2[(t,s32)/2, r, bh, (b4,s'32)] = 8c
                    cb2 = sm.tile([128, 2, 2, 128], FP8, tag="cb2")
                    for b in range(BC):
                        nc.vector.tensor_scalar_mul(
                            cb2[:, :, b // 4, (b % 4) * 32:(b % 4) * 32 + 32],
                            mask32_sb[:], crep[:, b:b + 1])

            # ---------- phase 3: DR mixing + de-interleave ----------
            # mxi2[(b4,s'32), bh, iclp, it, j]; per-sample mxcb tiles
            # [(icl2,u), it, j] so phase-4 sample b depends ONLY on its own
            # 4 de-interleaves (whole-tile dep would wait all 32)
            mxi2 = big.tile([128, 2, 4, KT, D], FP8, tag="mxi2")
            mxcb = []
            for b in range(BC):
                mxcb_b = big.tile([128, KT, D], FP8, tag=f"mxcb{b}")
                mxcb.append(mxcb_b)

            def deint(icl2, b, eng):
                eng.dma_start(
                    mxcb[b][icl2 * 32:(icl2 + 1) * 32, :, :],
                    mxi2[(b % 4) * 32:(b % 4) * 32 + 32, b // 4, icl2, :, :])

            with tc.tile_pool(name="psM", bufs=4, space="PSUM") as psM:
                for icl2 in range(4):
                    dwt = dw1_tiles[icl2]
                    for it in range(KT):
                        for bh in range(2):
                            pm2 = psM.tile([128, 2, 512], F32, tag="m")
                            for jh in range(2):
                                nc.tensor.matmul(
                                    pm2[:, jh, 0:384],
                                    cb2[:, :, bh, :],
                                    dwt[:, it * 2:it * 2 + 2,
                                        jh * 384:(jh + 1) * 384],
                                    start=True, stop=True,
                                    perf_mode=DR)
                            # 2-bank tiles, 4-deep ring: both engines drain
                            # back-to-back, decoupled from the MM stream
                            # (fp8 conversion ~1ns/elem sets the phase pace)
                            dstb = mxi2[:, bh, icl2, it, :].rearrange(
                                "p (b c) -> p b c", b=2, c=384)
                            if (icl2 * 12 + it * 2 + bh) % 2 == 0:
                                nc.scalar.mul(
                                    dstb[:], pm2[:, :, 0:384], 1.0)
                            else:
                                nc.vector.tensor_tensor(
                                    dstb[:], pm2[:, :, 0:384],
                                    zeros_sb[:, 0, :, :], op=ADD)
                    if icl2 < 3:
                        for b in range(BC):
                            deint(icl2, b, nc.sync if b < 4 else nc.gpsimd)
                    else:
                        # tail: b-ascending across three queues so phase 4
                        # (sample 0 first) starts ~1 DMA after last drain
                        for b in range(BC):
                            eng = (nc.sync, nc.gpsimd, nc.scalar)[
                                (0, 1, 2, 0, 1, 0, 1, 2)[b]]
                            deint(icl2, b, eng)

            # w2 + dw2 pair-tiles on gpsimd, gated on cb2 (ready at ph2
            # end) so their transfers don't steal DMA fabric from the
            # ph1-gating xt/w1 loads. Needed only by phase 5.
            w2_sb = big.tile([128, KT, D], BF16, tag="w2")
            nc.gpsimd.tensor_copy(w2_sb[0:1, 0, 0:4], cb2[0:1, 0, 0, 0:4])
            nc.gpsimd.dma_start(w2_sb[:], w2[:])
            dw2_tiles = []
            for pr in range(4):
                t_ = dwp.tile([128, 2 * KT, D], FP8, tag="dw")
                nc.gpsimd.tensor_copy(t_[0:1, 0, 0:4], cb2[0:1, 0, 0, 0:4])
                nc.gpsimd.dma_start(t_[:], dw2[pr])
                dw2_tiles.append(t_)

            # nb1t / cbc tiny matmuls: tucked into the de-interleave tail
            # where the PE would idle anyway (also keeps HAM warm).
            nb1t = sm.tile([128, KT, BC], F32, tag="nb1t")
            cbc = sm.tile([128, T, BC], BF16, tag="cbc")
            with tc.tile_pool(name="psT", bufs=1, space="PSUM") as psT:
                pt_ = psT.tile([128, 2, 64], F32, tag="t")
                for jt in range(KT):
                    nc.tensor.matmul(
                        pt_[:, 0, jt * 8:jt * 8 + 8],
                        db1_sb[:, jt * 128:(jt + 1) * 128],
                        coefsT_bf[:], start=(jt == 0), stop=(jt == KT - 1))
                for t in range(T):
                    nc.tensor.matmul(
                        pt_[:, 1, t * 8:t * 8 + 8],
                        selb_sb[:, t, :], coefsT_bf[:],
                        start=(t == 0), stop=(t == T - 1))
                nc.vector.tensor_copy(
                    nb1t[:].rearrange("p k b -> p (k b)"), pt_[:, 0, 0:48])
                nc.vector.tensor_copy(
                    cbc[:].rearrange("p t b -> p (t b)"), pt_[:, 1, :])

            # ---------- phase 4: final per-sample pass (fp8 normal) --------
            with tc.tile_pool(name="psF", bufs=2, space="PSUM") as psF:
                for b in range(BC):
                    pf = psF.tile([128, 3, 2, 256], F32, tag="f")
                    for jt in range(KT):
                        for it in range(KT):
                            nc.tensor.matmul(
                                pf[:, jt // 2, jt % 2, 0:196],
                                mxcb[b][:, it, jt * 128:(jt + 1) * 128],
                                xt_sb[:, it, b * NPAT:(b + 1) * NPAT],
                                start=(it == 0), stop=(it == KT - 1))
                    s_sb = scr.tile([128, KT, NPAT], BF16, tag="s")
                    ro4 = scr.tile([128, KT, NPAT], BF16, tag="ro4")
                    if b < BC - 1:
                        nc.vector.tensor_tensor(
                            s_sb[:].rearrange(
                                "p (a b) n -> p a b n", a=3, b=2),
                            pf[:, :, :, 0:196],
                            pa64[:, :, b, :].rearrange(
                                "p (a b) n -> p a b n", a=3, b=2),
                            op=ADD)
                        for jt in range(KT):
                            nc.scalar.activation(
                                ro4[:, jt, :], s_sb[:, jt, :], RELU,
                                bias=nb1t[:, jt, b:b + 1], scale=1.0 / SC)
                        nc.vector.tensor_reduce(
                            pooln[:, :, b], ro4[:], axis=AXX, op=ADD)
                    else:
                        # last sample: per-jt-pair pipeline ACT/DVE to cut
                        # the serial tail into phase 5 (keeps HAM warm too)
                        for jp in range(3):
                            nc.vector.tensor_tensor(
                                s_sb[:, 2 * jp:2 * jp + 2, :],
                                pf[:, jp, :, 0:196],
                                pa64[:, 2 * jp:2 * jp + 2, b, :],
                                op=ADD)
                            for jt in (2 * jp, 2 * jp + 1):
                                nc.scalar.activation(
                                    ro4[:, jt, :], s_sb[:, jt, :], RELU,
                                    bias=nb1t[:, jt, b:b + 1],
                                    scale=1.0 / SC)
                            nc.vector.tensor_reduce(
                                pooln[:, 2 * jp:2 * jp + 2, b],
                                ro4[:, 2 * jp:2 * jp + 2, :],
                                axis=AXX, op=ADD)
                            # HAM keeper: a dummy matmul gated on this
                            # pair's relu pulses the PE through the tail so
                            # phase 5 starts at full clock
                            pfD = psF.tile([8, 256], F32, tag="fd")
                            nc.tensor.matmul(
                                pfD[:, 0:196], ro4[:, 2 * jp, 0:8],
                                ro4[:, 2 * jp, :], start=True, stop=True)
                            nc.tensor.matmul(
                                pfD[:, 0:196], ro4[:, 2 * jp + 1, 0:8],
                                ro4[:, 2 * jp + 1, :], start=True, stop=True)

            # ---------- phase 5: layer 2 ----------
            pooln_bf = sm.tile([128, KT, BC], BF16, tag="poolnbf")
            nc.vector.tensor_copy(pooln_bf[:], pooln[:])

            out_sb = sm.tile([BC, D], F32, tag="out")
            tdelt = sm.tile([BC, D], F32, tag="tdelt")
            with tc.tile_pool(name="psV", bufs=4, space="PSUM") as psV:
                po0 = psV.tile([8, 384], F32, tag="v")
                po1 = psV.tile([8, 384], F32, tag="v")
                po = [po0, po1]
                pd0 = psV.tile([16, 384], F32, tag="v8")
                pd1 = psV.tile([16, 384], F32, tag="v8")
                pd = [pd0, pd1]
                # main chain first (overlaps the uall build below)
                for eh in range(2):
                    for kt in range(KT):
                        nc.tensor.matmul(
                            po[eh][:], pooln_bf[:, kt, :],
                            w2_sb[:, kt, eh * 384:(eh + 1) * 384],
                            start=(kt == 0), stop=False)
                    nc.tensor.matmul(po[eh][:], coefsT_bf[:],
                                     db2_sb[:, eh * 384:(eh + 1) * 384],
                                     start=False, stop=True)

                # U[(t,kt)][j_local, b] = (64/196) c[b,t] * pooln[b, .]
                for t in range(T):
                    for kt in range(KT):
                        eng = nc.vector if kt < 3 else nc.gpsimd
                        eng.tensor_tensor(
                            uall[:, t, kt, 0:BC],
                            pooln_bf[:, kt, :],
                            cbc[:, t, :], op=MULT)

                # delta chain: DoubleRow over (t, kt-pairs)
                for t in range(T):
                    dwt2 = dw2_tiles[t // 2]
                    tp = t % 2
                    for g in range(3):
                        for eh in range(2):
                            nc.tensor.matmul(
                                pd[eh][:],
                                uall[:, t, 2 * g:2 * g + 2, :],
                                dwt2[:, tp * KT + 2 * g:tp * KT + 2 * g + 2,
                                     eh * 384:(eh + 1) * 384],
                                start=(t == 0 and g == 0),
                                stop=(t == T - 1 and g == 2),
                                perf_mode=DR)
                for eh in range(2):
                    nc.scalar.mul(
                        tdelt[:, eh * 384:(eh + 1) * 384],
                        pd[eh][0:BC, :], 1.0 / (SC * SC))
                    nc.vector.tensor_tensor(
                        out_sb[:, eh * 384:(eh + 1) * 384], po[eh][:],
                        b2r_sb[:, eh * 384:(eh + 1) * 384],
                        op=ADD)
                    nc.vector.tensor_tensor(
                        out_sb[:, eh * 384:(eh + 1) * 384],
                        out_sb[:, eh * 384:(eh + 1) * 384],
                        tdelt[:, eh * 384:(eh + 1) * 384],
                        op=ADD)
                nc.sync.dma_start(out[:], out_sb[:])
                if _DEBUG_TAPS:
                    nc.vector.tensor_copy(poolb[:], poolb_bf[:])
                    nc.sync.dma_start(dbg_poolb[:], poolb[:])
                    nc.sync.dma_start(dbg_coefs[:], coefsT[:])
                    nc.sync.dma_start(dbg_pooln[:], pooln[:])
                    nc.sync.dma_start(dbg_tdelt[:], tdelt[:])
                    for b in range(BC):
                        nc.sync.dma_start(dbg_mxc[:, b, :, :], mxcb[b][:])
                    nc.sync.dma_start(dbg_pa64[:], pa64[:])

    _split_multi_waits(nc)
    return nc


def prep_inputs(x, W1, b1, W2, b2, dW1, db1, dW2, db2, mw1, mb1, mw2, mb2):
    """Host-side layout prep. Returns per-core in_maps."""
    bf = ml_dtypes.bfloat16
    f8 = ml_dtypes.float8_e4m3
    x = np.asarray(x); W1 = np.asarray(W1); W2 = np.asarray(W2)
    b1 = np.asarray(b1); b2 = np.asarray(b2)
    dW1 = np.asarray(dW1); dW2 = np.asarray(dW2)
    db1 = np.asarray(db1); db2 = np.asarray(db2)
    mw1 = np.asarray(mw1); mb1 = np.asarray(mb1)
    mw2 = np.asarray(mw2); mb2 = np.asarray(mb2)

    # patches^T: [B, D, NPAT]
    pt = x.reshape(B, 3, 14, P, 14, P).transpose(0, 1, 3, 5, 2, 4)
    pt = np.ascontiguousarray(pt).reshape(B, D, NPAT)

    w1_c = np.ascontiguousarray(
        (W1 * SC).reshape(KT, 128, D).transpose(1, 0, 2)).astype(f8)
    w2_c = np.ascontiguousarray(
        (W2 / NPAT).reshape(KT, 128, D).transpose(1, 0, 2)).astype(bf)

    # dw1 pair-tiles: i = (it*4 + icl2)*32 + u; contract row c = t*32+u,
    # moving element [p, r] with c = 2p+r.
    d1 = (dW1 * SCC).reshape(T, KT, 4, 32, D)        # [t, it, icl2, u, j]
    # -> c-major [c=(t,u), icl2, it, j], c = t*32 + u = 2p + r
    d1 = np.ascontiguousarray(d1.transpose(0, 3, 2, 1, 4)).reshape(
        256, 4, KT, D)
    # [c, icl2, it, j] -> [icl2, p, (it, r), j]
    d1 = d1.reshape(128, 2, 4, KT, D).transpose(2, 0, 3, 1, 4)
    dw1_c = np.ascontiguousarray(d1.reshape(4, 128, KT * 2, D)).astype(f8)

    # dw2 pair-tiles: [pr, j_local, (tp, kt), e]
    d2 = (dW2 * SC).reshape(4, 2, KT, 128, D)        # [pr, tp, kt, j, e]
    dw2_c = np.ascontiguousarray(
        d2.transpose(0, 3, 1, 2, 4).reshape(4, 128, 2 * KT, D)).astype(f8)

    b1t64_c = np.ascontiguousarray((b1 * SC).reshape(KT, 128).T).astype(
        np.float32)
    mwf = (W2.astype(np.float32) @ mw1.astype(np.float32)) / (SC * NPAT)
    mbf = b2.astype(np.float32) @ mw1.astype(np.float32) + mb1
    mb1t_c = np.zeros((128, 2), np.float32)
    mb1t_c[:, 0] = mbf[:128]
    mb1t_c[:64, 1] = mbf[128:]
    # mask32[p, r, s'] = 8 * (u(p,r) == s'), u = (2p+r) % 32
    m32 = np.zeros((128, 2, 32), np.float32)
    for p_ in range(128):
        for r_ in range(2):
            m32[p_, r_, (2 * p_ + r_) % 32] = SCC
    mask32_c = m32.astype(bf)
    mw2_c = np.zeros((128, 2, T), np.float32)
    mw2_c[:, 0, :] = mw2[:128]
    mw2_c[:64, 1, :] = mw2[128:]
    mw2_c = mw2_c.astype(bf)
    mw1_c = np.ascontiguousarray(
        mwf.reshape(KT, 128, HM).transpose(1, 0, 2)).astype(bf)
    iexp_c = np.repeat(np.eye(T, dtype=np.float32), P, axis=1)
    mb2t_c = mb2.reshape(T, 1).astype(np.float32)
    selb_c = np.ascontiguousarray(np.broadcast_to(
        ((SC / NPAT) * np.eye(T, dtype=np.float32))[:, :, None],
        (T, T, 128))).astype(bf)
    db1_c = db1.astype(bf)
    db2_c = db2.astype(bf)
    b2r_c = np.broadcast_to(b2.astype(np.float32), (BC, D)).copy()

    shared = dict(w1=w1_c, w2=w2_c, dw1=dw1_c, dw2=dw2_c,
                  b1t64=b1t64_c, mb1t=mb1t_c, mask32=mask32_c, mw2=mw2_c,
                  mw1=mw1_c, iexp=iexp_c, mb2t=mb2t_c, selb=selb_c,
                  db1=db1_c, db2=db2_c, b2r=b2r_c)

    in_maps = []
    for c in range(NCORES):
        ptc = pt[c * BC:(c + 1) * BC]                  # [BC, D, NPAT]
        xt_c = np.ascontiguousarray(
            ptc.reshape(BC, KT, 128, NPAT).transpose(2, 1, 0, 3)
        ).reshape(128, KT, NB).astype(f8)
        m = dict(shared)
        m["xt"] = xt_c
        in_maps.append(m)
    return in_maps


_NC_CACHE = {}


def kernel(**inputs) -> np.ndarray:
    _apply_tile_patch()
    if "nc" not in _NC_CACHE:
        _NC_CACHE["nc"] = build_kernel()
    nc = _NC_CACHE["nc"]
    in_maps = prep_inputs(**inputs)
    res = run_bass_kernel_spmd(nc, in_maps, core_ids=list(range(NCORES)))
    return np.concatenate([r["out"] for r in res.results], axis=0)
